# revision 25
# baseline (speedup 1.0000x reference)
"""CardAwarePolicy Trainium2 kernel (8-core data parallel), v2.

Design notes:
- All weight-derived tables are precomputed host-side (numpy) in kernel()
  and passed to the device as extra DRAM parameters; the device does only
  the per-sample work.
- Attention collapses into card-id space (q/k/v of a card depend only on
  its id).  Per-sample card histogram h[c] is computed ON THE PE via an
  exact one-hot matmul: 1[x==c] = relu(1 - (xh-ch)^2 - (xl-cl)^2) where
  x = 8*xh + xl (3-bit digit split keeps every product an integer <= 98,
  exact in bf16).  Features [1, xh, xh^2, xl, xl^2] per position are
  transposed once per 128-batch tile; 4 matmuls (2 positions x 64 cards
  packed into 128 partitions) + relu + a summing matmul give h.
- Attention with UNSCALED h: den[ci] = sum_{cj!=0} E[ci,cj] h[cj] (zeroed
  rows in the den table), W = h * recip(den)  (the 1/len cancels), S with
  the cj=0 column zeroed in the S tables, U = S * (h * rlT) where
  rlT[p,b] = 1/max(hand_size,1) is materialized by a rank-1 matmul.
  An augmented 65th value row carries (8/len)*out_b through the fused
  (c1_w @ out_w) projection.
- Then feature-major MLPs, and the 20-action scorer:
  h1 = relu(P1 + Ptab[a]) (bf16, DVE 2x mode / ACT), block-diag s2
  matmuls, and s3 as 5 accumulating matmuls with a [128,32] zero-padded
  w3 selector (output [32, 512] scores.T, PE-transposed back).
- Nothing in the main loop runs on GPSIMD (its per-op overhead measured
  ~2.9us here).  ScalarE uses only {reciprocal, relu, copy} = one table
  set, so no ACT table reloads inside the loop.
"""
import sys

if "/opt/trn_rl_repo" not in sys.path:
    sys.path.insert(0, "/opt/trn_rl_repo")

import numpy as np
import ml_dtypes
from contextlib import ExitStack

import concourse.bass as bass
import concourse.tile as tile
from concourse import mybir

F32 = mybir.dt.float32
BF = mybir.dt.bfloat16
I32 = mybir.dt.int32
R = mybir.dt.float32r
OP = mybir.AluOpType
AF = mybir.ActivationFunctionType

B, S, A, KC = 32768, 8, 20, 4
E, H, D = 64, 4, 16
NCARD = 54
NCORES = 8
BC = B // NCORES  # 4096 per core
NC2 = 64          # padded card space
BF16 = ml_dtypes.bfloat16


def legalize_multiwait(nc):
    """Split >1 sem waits on Drain/CTRL instructions (walrus limit) into
    preceding single-wait EventSemaphore carriers."""
    for fn in nc.m.functions:
        for blk in fn.blocks:
            new_list = []
            for inst in blk.instructions:
                si = inst.sync_info
                if si and si.on_wait and len(si.on_wait) > 1:
                    waits = list(si.on_wait)
                    for w in waits[:-1]:
                        nm = f"{inst.name}-wsplit-{w.id}"
                        d = mybir.InstEventSemaphore(name=nm, ins=[], outs=[])
                        d.engine = inst.engine
                        d.sync_info = mybir.SyncInfo(on_wait=[w], on_update=[])
                        nc.register_instruction(d, overwrite=True)
                        new_list.append(d)
                    si.on_wait[:] = [waits[-1]]
                new_list.append(inst)
            blk.instructions[:] = new_list
    return nc


def make_tables(inputs):
    """Host-side (numpy) computation of every weight-derived constant."""
    f32 = np.float32
    g = lambda k: np.asarray(inputs[k], f32)
    emb, ipw, ipb = g("emb"), g("in_proj_w"), g("in_proj_b")
    ow, ob = g("out_w"), g("out_b")
    g1w, g1b = g("g1_w"), g("g1_b")
    g2w, g2b = g("g2_w"), g("g2_b")
    c1w, c1b = g("c1_w"), g("c1_b")
    c2w, c2b = g("c2_w"), g("c2_b")
    s1w, s1b = g("s1_w"), g("s1_b")
    s2w, s2b = g("s2_w"), g("s2_b")
    s3w, s3b = g("s3_w"), g("s3_b")
    aci = np.asarray(inputs["action_card_indices"], np.int64)
    acc = np.asarray(inputs["action_card_counts"], np.int64)

    embp = np.zeros((NC2, E), f32)
    embp[:NCARD] = emb
    q = embp @ ipw[0:64].T + ipb[0:64]       # [64c, 64]
    k = embp @ ipw[64:128].T + ipb[64:128]
    v = embp @ ipw[128:192].T + ipb[128:192]

    Eh = np.zeros((H, NC2, NC2), f32)        # [h, ci, cj]
    for h in range(H):
        sc = (q[:, h * D:(h + 1) * D] @ k[:, h * D:(h + 1) * D].T) / np.sqrt(
            np.float32(D))
        Eh[h] = np.exp(sc)

    # den tables: DenT[g][cj, u*64+ci] = Eh[2g+u][ci, cj]; zero cj=0, cj>=54
    DenT = np.zeros((NC2, 2, 128), f32)
    for gg in range(2):
        for u in range(2):
            DenT[:, gg, u * 64:(u + 1) * 64] = Eh[2 * gg + u].T
    DenT[0, :, :] = 0.0
    DenT[NCARD:, :, :] = 0.0
    DenT = DenT.reshape(NC2, 256)

    # S tables (bf16, E-1 shift): BD1z ones-blocks + SE blockdiag(E-1);
    # total column (u, cj=0) must be zero -> zero it in both.
    BD1z = np.zeros((128, 128), f32)
    SE = np.zeros((128, 2, 128), f32)
    for u in range(2):
        BD1z[u * 64:(u + 1) * 64, u * 64:(u + 1) * 64] = 1.0
        BD1z[:, u * 64] = 0.0
        for gg in range(2):
            SE[u * 64:(u + 1) * 64, gg, u * 64:(u + 1) * 64] = Eh[2 * gg + u] - 1.0
            SE[:, gg, u * 64] = 0.0
    SE = SE.reshape(128, 256)

    # BDVP[g] [128, 65]: rows (u,cj) -> v cols of head 2g+u; aug col 64
    BDVP = np.zeros((128, 2, E + 1), f32)
    for gg in range(2):
        for u in range(2):
            hh = 2 * gg + u
            BDVP[u * 64:(u + 1) * 64, gg, hh * D:(hh + 1) * D] = \
                v[:, hh * D:(hh + 1) * D]
    BDVP[0:64, 0, E] = 1.0
    BDVP = BDVP.reshape(128, 2 * (E + 1))

    # fused (c1_w[:, :64] @ out_w_aug): hgw [97, 128]
    c1A = c1w[:, 0:64]
    M1T = np.zeros((E + 1, 128), f32)
    M1T[0:E] = (c1A @ ow).T
    M1T[E] = c1A @ ob
    c1wB = c1w[:, 64:96].T                                  # [32, 128]

    # one-hot quadratic tables CP [4 pairs, 33, 128] (bf16-exact ints)
    cs = np.arange(NC2)
    ch, cl = cs >> 3, cs & 7
    CP = np.zeros((4, 33, 128), f32)
    for p in range(4):
        for u in range(2):
            j = 2 * p + u
            col = slice(u * 64, (u + 1) * 64)
            CP[p, 0, col] = 1.0 - ch * ch - cl * cl
            CP[p, 1 + j, col] = 2.0 * ch
            CP[p, 9 + j, col] = -1.0
            CP[p, 17 + j, col] = 2.0 * cl
            CP[p, 25 + j, col] = -1.0
    CPt = CP.transpose(1, 0, 2).reshape(33, 512)

    ones22I = np.tile(np.eye(NC2, dtype=f32), (2, 2))       # [128, 128]
    onesrow = np.ones((1, 128), f32)
    # merged S table (single matmul per g): blockdiag(E) with cj=0 col zeroed
    SEF = np.zeros((128, 2, 128), f32)
    for u in range(2):
        for gg in range(2):
            SEF[u * 64:(u + 1) * 64, gg, u * 64:(u + 1) * 64] = Eh[2 * gg + u]
            SEF[:, gg, u * 64] = 0.0
    SEF = SEF.reshape(128, 256)

    # actions
    ae = embp[np.clip(aci, 0, NC2 - 1)]                     # [A, KC, E]
    amask = (np.arange(KC)[None, :] < acc[:, None]).astype(f32)
    arep = (ae * amask[..., None]).sum(axis=1) / np.maximum(acc, 1)[:, None]
    Ptab = s1w[:, 128:192] @ arep.T + s1b[:, None]          # [64, A]
    Ptab2 = np.zeros((128, A // 2), f32)
    Ptab2[0:64] = Ptab[:, 0::2]
    Ptab2[64:128] = Ptab[:, 1::2]

    s1A = s1w[:, 0:128]                                     # [64, 128]
    s1wAd = np.concatenate([s1A.T, s1A.T], axis=1)          # [128, 128]

    BDs2 = np.zeros((128, 2, 128), f32)
    s2wT = s2w.T                                            # [64, 32]
    for half in range(2):
        BDs2[0:64, half, half * 64:half * 64 + 32] = s2wT
        BDs2[64:128, half, half * 64 + 32:half * 64 + 64] = s2wT
    BDs2 = BDs2.reshape(128, 256)

    W3 = np.zeros((5, 128, 32), f32)
    for jj in range(5):
        for u in range(4):
            W3[jj, u * 32:(u + 1) * 32, 4 * jj + u] = s3w[0]
    W3t = W3.transpose(1, 0, 2).reshape(128, 160)

    biases = np.zeros((128, 8), f32)
    biases[0:64, 0] = g1b
    biases[0:32, 1] = g2b
    biases[:, 2] = c1b
    biases[:, 3] = c2b
    biases[:, 4] = np.tile(s2b, 4)
    biases[:, 5] = s3b[0]

    bf = lambda x: np.ascontiguousarray(x.astype(BF16))
    fl = lambda x: np.ascontiguousarray(x.astype(f32))
    return dict(
        identF=fl(np.eye(128)),
        identB=bf(np.eye(128)),
        cpt=bf(CPt),
        ones22I=bf(ones22I),
        onesrow=fl(onesrow),
        dent=bf(DenT),
        bd1z=bf(BD1z),
        se=bf(SE),
        sef=bf(SEF),
        bdvp=bf(BDVP),
        m1t=bf(M1T),
        c1wB=bf(c1wB),
        g1wT=bf(g1w.T),
        g2wT=bf(g2w.T),
        c2wT=bf(c2w.T),
        s1wAd=bf(s1wAd),
        ptab2f=fl(Ptab2),
        ptab2b=bf(Ptab2),
        bds2=bf(BDs2),
        s2wTt=bf(np.concatenate([s2wT, s2wT], axis=0)),
        w3t=fl(W3t),
        biases=fl(biases),
    )


TABLE_SPECS = [
    ("identF", [128, 128], F32), ("identB", [128, 128], BF),
    ("cpt", [33, 512], BF), ("ones22I", [128, 128], BF),
    ("onesrow", [1, 128], F32), ("dent", [64, 256], BF),
    ("bd1z", [128, 128], BF), ("se", [128, 256], BF),
    ("sef", [128, 256], BF),
    ("bdvp", [128, 130], BF), ("m1t", [65, 128], BF),
    ("c1wB", [32, 128], BF),
    ("g1wT", [12, 64], BF), ("g2wT", [64, 32], BF),
    ("c2wT", [128, 128], BF), ("s1wAd", [128, 128], BF),
    ("ptab2f", [128, 10], F32), ("ptab2b", [128, 10], BF),
    ("bds2", [128, 256], BF), ("s2wTt", [128, 32], BF),
    ("w3t", [128, 160], F32),
    ("biases", [128, 8], F32),
]


def build_nc(b_core=BC, nva=A, reps=1, n_h1_act=1, n_h2_act=4, oh_act=2,
             se_merge=True):
    CH = 512
    assert b_core % CH == 0
    nchunks = b_core // CH
    NT = CH // 128

    nc = bass.Bass()
    dp = nc.declare_dram_parameter
    # host-swizzled layouts: every DMA is a contiguous [128, X] block
    cards_d = dp("hand_cards", [b_core // NT, NT * S], I32, isOutput=False)
    gs_d = dp("game_state", [12, b_core], BF, isOutput=False)
    rl_d = dp("rl_row", [b_core // CH, CH], F32, isOutput=False)
    tbl_d = {name: dp(name, shape, dt, isOutput=False)
             for name, shape, dt in TABLE_SPECS}
    out_d = dp("out", [b_core // NT, NT * A], F32, isOutput=True)

    with tile.TileContext(nc) as tc:
        with ExitStack() as ctx:
            const = ctx.enter_context(tc.tile_pool(name="const", bufs=1))

            _dmae = [nc.sync, nc.scalar, nc.gpsimd]
            _dmac = [0]

            def cdma(**kw):
                e = _dmae[_dmac[0] % len(_dmae)]
                _dmac[0] += 1
                e.dma_start(**kw)

            # ---- constants (R tiles are DMA'd through a f32 bitcast) ----
            ct = {}
            r_tiles = {"onesrow", "w3t"}
            for name, shape, dt in TABLE_SPECS:
                if name in r_tiles:
                    stage = const.tile(shape, F32, name="s_" + name,
                                       tag="s_" + name)
                    cdma(out=stage, in_=tbl_d[name][:, :])
                    t = const.tile(shape, R, name="c_" + name, tag="c_" + name)
                    nc.vector.tensor_copy(t, stage)
                else:
                    t = const.tile(shape, dt, name="c_" + name, tag="c_" + name)
                    cdma(out=t, in_=tbl_d[name][:, :])
                ct[name] = t
            identF = ct["identF"]
            identB = ct["identB"]

            # ---- per-chunk pipeline ----
            sb = ctx.enter_context(tc.tile_pool(name="sb", bufs=3))
            sbh = ctx.enter_context(tc.tile_pool(name="sbh", bufs=4))
            sbo = ctx.enter_context(tc.tile_pool(name="sbo", bufs=4))
            sbh2 = ctx.enter_context(tc.tile_pool(name="sbh2", bufs=7))
            ps1 = ctx.enter_context(tc.tile_pool(name="ps1", bufs=3, space="PSUM"))
            psACC = ctx.enter_context(tc.tile_pool(name="psACC", bufs=3, space="PSUM"))
            psM = ctx.enter_context(tc.tile_pool(name="psM", bufs=2, space="PSUM"))

            cards_r = cards_d.rearrange("(c p) x -> c p x", p=128)
            gsT_r = gs_d.rearrange("f (c b) -> c f b", b=CH)
            out_r = out_d.rearrange("(c p) x -> c p x", p=128)

            def stageA(cix):
                """loads + histogram + attention + g-MLP -> (hsum, g2s)."""
                # ---------- loads ----------
                c4 = sb.tile([128, NT, S], I32, tag="c4")
                nc.sync.dma_start(out=c4.rearrange("p t s -> p (t s)"),
                                  in_=cards_r[cix])
                gsT = sb.tile([12, CH], BF, tag="gsT")
                nc.sync.dma_start(out=gsT, in_=gsT_r[cix])
                rlrow_f = sb.tile([1, CH], F32, tag="rlrow_f")
                nc.scalar.dma_start(out=rlrow_f, in_=rl_d[cix].unsqueeze(0))

                # ---------- 1/len row -> rlT [128, CH] ----------
                rlrow = sb.tile([1, CH], R, tag="rlrow")
                nc.vector.tensor_copy(rlrow, rlrow_f)
                rlT_ps = ps1.tile([128, CH], F32, tag="p1", name="rlT_ps")
                nc.tensor.matmul(rlT_ps, ct["onesrow"], rlrow, start=True, stop=True)
                rlT = sbh.tile([128, CH], R, tag="rlT")
                nc.vector.tensor_copy(rlT, rlT_ps)

                # ---------- one-hot features ----------
                xh = sb.tile([128, NT, S], I32, tag="xh")
                nc.vector.tensor_scalar(xh, c4, 3, None, OP.arith_shift_right)
                xl = sb.tile([128, NT, S], I32, tag="xl")
                nc.vector.tensor_scalar(xl, c4, 7, None, OP.bitwise_and)
                Fall = sb.tile([128, NT, 33], BF, tag="Fall")
                nc.vector.memset(Fall[:, :, 0:1], 1.0)
                nc.vector.tensor_copy(Fall[:, :, 1:9], xh)
                nc.vector.tensor_copy(Fall[:, :, 17:25], xl)
                nc.vector.tensor_tensor(out=Fall[:, :, 9:17], in0=Fall[:, :, 1:9],
                                        in1=Fall[:, :, 1:9], op=OP.mult)
                nc.vector.tensor_tensor(out=Fall[:, :, 25:33], in0=Fall[:, :, 17:25],
                                        in1=Fall[:, :, 17:25], op=OP.mult)
                F_ps = ps1.tile([33, CH], BF, tag="p1", name="F_ps")
                for t in range(NT):
                    nc.tensor.transpose(F_ps[:, t * 128:(t + 1) * 128],
                                        Fall[:, t, :], identB)
                F_s = sbh.tile([33, CH], BF, tag="F_s")
                nc.vector.tensor_copy(F_s, F_ps)

                # ---------- histogram: 4 pair-matmuls + relu + sum ----------
                h2_ps = psACC.tile([128, CH], F32, tag="pA", name="h2_ps")
                OHs = []
                for p in range(4):
                    OH_ps = ps1.tile([128, CH], F32, tag="p1", name=f"OH{p}")
                    nc.tensor.matmul(OH_ps, ct["cpt"][:, p * 128:(p + 1) * 128],
                                     F_s, start=True, stop=True)
                    OH_s = sbh.tile([128, CH], BF, tag="OH_s", name=f"OHs{p}")
                    if p < oh_act:
                        nc.scalar.activation(OH_s, OH_ps, AF.Relu)
                    else:
                        nc.vector.tensor_scalar_max(OH_s, OH_ps, 0.0)
                    OHs.append(OH_s)
                for p in range(4):
                    nc.tensor.matmul(h2_ps, ct["ones22I"], OHs[p],
                                     start=(p == 0), stop=(p == 3))
                h2s = sbh.tile([128, CH], BF, tag="h2s")
                nc.scalar.activation(h2s, h2_ps, AF.Copy)
                h2nzs = sbh.tile([128, CH], R, tag="h2nzs")
                nc.vector.tensor_tensor(out=h2nzs, in0=h2_ps,
                                        in1=rlT.bitcast(F32), op=OP.mult)

                # ---------- attention in card space ----------
                hs_ps = psACC.tile([E + 1, CH], F32, tag="pA", name="hs_ps")
                for g in range(2):
                    den_ps = ps1.tile([128, CH], F32, tag="p1", name=f"den{g}")
                    nc.tensor.matmul(den_ps, ct["dent"][:, g * 128:(g + 1) * 128],
                                     h2s[0:64, :], start=True, stop=True)
                    lden = sbh.tile([128, CH], F32, tag="lden")
                    nc.scalar.activation(lden, den_ps, AF.Ln)
                    rden = sbh.tile([128, CH], F32, tag="rden")
                    nc.scalar.activation(rden, lden, AF.Exp, scale=-1.0)
                    Wg = sbh.tile([128, CH], BF, tag="Wg")
                    nc.vector.tensor_tensor(out=Wg, in0=rden,
                                            in1=h2s, op=OP.mult)
                    S_ps = ps1.tile([128, CH], F32, tag="p1", name=f"S{g}")
                    if se_merge:
                        nc.tensor.matmul(S_ps, ct["sef"][:, g * 128:(g + 1) * 128],
                                         Wg, start=True, stop=True)
                    else:
                        nc.tensor.matmul(S_ps, ct["bd1z"], Wg, start=True,
                                         stop=False)
                        nc.tensor.matmul(S_ps, ct["se"][:, g * 128:(g + 1) * 128],
                                         Wg, start=False, stop=True)
                    Ug = sbh.tile([128, CH], BF, tag="Ug")
                    nc.vector.tensor_tensor(out=Ug, in0=S_ps,
                                            in1=h2nzs.bitcast(F32), op=OP.mult)
                    nc.tensor.matmul(hs_ps, ct["bdvp"][:, g * 65:(g + 1) * 65],
                                     Ug, start=(g == 0), stop=(g == 1))

                # ---------- g-MLP + handsum evac ----------
                bia = ct["biases"]
                hsum = sbh.tile([65, CH], BF, tag="hsum")
                nc.scalar.activation(hsum, hs_ps, AF.Copy)
                g1_ps = psM.tile([64, CH], F32, tag="pM", name="g1_ps")
                nc.tensor.matmul(g1_ps, ct["g1wT"], gsT, start=True, stop=True)
                g1s = sbh.tile([64, CH], BF, tag="g1s")
                nc.scalar.activation(g1s, g1_ps, AF.Relu, bias=bia[0:64, 0:1])
                g2_ps = psM.tile([32, CH], F32, tag="pM", name="g2_ps")
                nc.tensor.matmul(g2_ps, ct["g2wT"], g1s, start=True, stop=True)
                g2s = sbh.tile([32, CH], BF, tag="g2s")
                nc.scalar.activation(g2s, g2_ps, AF.Relu, bias=bia[0:32, 1:2])
                return cix, hsum, g2s

            def stageB(state):
                """context MLP + action scorer + store for a prior chunk."""
                cix, hsum, g2s = state
                bia = ct["biases"]
                c1_ps = psM.tile([128, CH], F32, tag="pM", name="c1_ps")
                nc.tensor.matmul(c1_ps, ct["m1t"], hsum, start=True, stop=False)
                nc.tensor.matmul(c1_ps, ct["c1wB"], g2s, start=False, stop=True)
                ctx1 = sbh.tile([128, CH], BF, tag="ctx1")
                nc.scalar.activation(ctx1, c1_ps, AF.Relu, bias=bia[:, 2:3])
                c2_ps = psM.tile([128, CH], F32, tag="pM", name="c2_ps")
                nc.tensor.matmul(c2_ps, ct["c2wT"], ctx1, start=True, stop=True)
                ctx2 = sbh.tile([128, CH], BF, tag="ctx2")
                nc.scalar.activation(ctx2, c2_ps, AF.Relu, bias=bia[:, 3:4])
                p1_ps = psM.tile([128, CH], F32, tag="pM", name="p1_ps")
                nc.tensor.matmul(p1_ps, ct["s1wAd"], ctx2, start=True, stop=True)
                P1d = sbh.tile([128, CH], BF, tag="P1d")
                nc.vector.tensor_copy(P1d, p1_ps)

                # ---------- actions ----------
                scST_ps = psACC.tile([32, CH], F32, tag="pA", name="scST_ps")
                h1s = []
                for j in range(10):
                    h1 = sbh2.tile([128, CH], BF, tag="h1", name=f"h1_{j}")
                    if (j % 10) < n_h1_act:
                        nc.scalar.activation(h1, P1d, AF.Relu,
                                             bias=ct["ptab2f"][:, j:j + 1])
                    else:
                        nc.vector.tensor_scalar(
                            h1, P1d, ct["ptab2f"][:, j:j + 1], 0.0,
                            OP.add, OP.max)
                    h1s.append(h1)
                for jj in range(5):
                    hp = psM.tile([128, CH], F32, tag="pM", name=f"hp{jj}")
                    for u in range(4):
                        src_h1 = h1s[jj * 2 + u // 2]
                        half = u % 2
                        nc.tensor.matmul(
                            hp[u * 32:(u + 1) * 32, :],
                            ct["s2wTt"][half * 64:(half + 1) * 64, :],
                            src_h1[half * 64:(half + 1) * 64, :],
                            start=True, stop=True,
                            tile_position=(half * 64, u * 32))
                    h2t = sbh2.tile([128, CH], R, tag="h2t", name=f"h2t{jj}")
                    if jj < n_h2_act:
                        nc.scalar.activation(h2t, hp, AF.Relu, bias=bia[:, 4:5])
                    else:
                        nc.vector.tensor_scalar(h2t, hp,
                                                bia[:, 4:5], 0.0, OP.add, OP.max)
                    nc.tensor.matmul(scST_ps, ct["w3t"][:, jj * 32:(jj + 1) * 32],
                                     h2t, start=(jj == 0), stop=(jj == 4))
                scS = sbh.tile([32, CH], R, tag="scS")
                nc.scalar.activation(scS, scST_ps, AF.Copy)
                so = sbo.tile([128, NT, A], F32, tag="so")
                for t in range(NT):
                    sot_ps = psM.tile([128, 32], F32, tag="pM", name=f"sot{t}")
                    nc.tensor.transpose(sot_ps,
                                        scS.bitcast(F32)[:, t * 128:(t + 1) * 128],
                                        identF[0:32, 0:32])
                    nc.vector.tensor_scalar_add(so[:, t, :], sot_ps[:, 0:A],
                                                bia[:, 5:6])
                if nva < A:
                    nc.vector.memset(so[:, :, nva:A], -1e8)
                nc.scalar.dma_start(out=out_r[cix], in_=so)

            # software-pipelined emission: chunk c's stageA interleaves with
            # chunk c-1's stageB so the PE always has independent work
            prev = None
            for cix in [c for _ in range(reps) for c in range(nchunks)]:
                st = stageA(cix)
                if prev is not None:
                    stageB(prev)
                prev = st
            stageB(prev)

    legalize_multiwait(nc)
    return nc


_NC_CACHE = {}


def _get_nc(b_core, nva):
    key = (b_core, nva)
    if key not in _NC_CACHE:
        _NC_CACHE[key] = build_nc(b_core=b_core, nva=nva)
    return _NC_CACHE[key]


# ---- cached PJRT execution path (avoids per-call retrace/re-lowering) ----
_EXEC_CACHE = {}


def _get_executor(nc):
    """Build (once) a jitted shard_map callable around the bass_exec
    primitive for `nc`, so repeated kernel() calls skip retracing and
    re-lowering.  Mirrors concourse.bass2jax.run_bass_via_pjrt."""
    key = id(nc)
    if key in _EXEC_CACHE:
        return _EXEC_CACHE[key]
    import jax
    from jax.sharding import Mesh, PartitionSpec
    from jax.experimental.shard_map import shard_map
    from concourse import bass2jax
    from concourse.bass2jax import _bass_exec_p, install_neuronx_cc_hook

    install_neuronx_cc_hook()
    partition_name = (nc.partition_id_tensor.name
                      if nc.partition_id_tensor else None)
    in_names, out_names, out_avals, zero_shapes = [], [], [], []
    for alloc in nc.m.functions[0].allocations:
        if not isinstance(alloc, mybir.MemoryLocationSet):
            continue
        name = alloc.memorylocations[0].name
        if alloc.kind == "ExternalInput":
            if name != partition_name:
                in_names.append(name)
        elif alloc.kind == "ExternalOutput":
            out_names.append(name)
            shape = tuple(alloc.tensor_shape)
            dtype = mybir.dt.np(alloc.dtype)
            out_avals.append(jax.core.ShapedArray(shape, dtype))
            zero_shapes.append((shape, dtype))
    n_params = len(in_names)
    n_outs = len(out_names)
    all_names = in_names + out_names
    if partition_name is not None:
        all_names = all_names + [partition_name]
    donate = tuple(range(n_params, n_params + n_outs))

    def _body(*args):
        operands = list(args)
        if partition_name is not None:
            operands.append(bass2jax.partition_id_tensor())
        outs = _bass_exec_p.bind(
            *operands,
            out_avals=tuple(out_avals),
            in_names=tuple(all_names),
            out_names=tuple(out_names),
            lowering_input_output_aliases=(),
            sim_require_finite=True,
            sim_require_nnan=True,
            nc=nc,
        )
        return tuple(outs)

    devices = jax.devices()[:NCORES]
    mesh = Mesh(np.asarray(devices), ("core",))
    in_specs = (PartitionSpec("core"),) * (n_params + n_outs)
    out_specs = (PartitionSpec("core"),) * n_outs
    sharded = jax.jit(
        shard_map(_body, mesh=mesh, in_specs=in_specs, out_specs=out_specs,
                  check_rep=False),
        donate_argnums=donate, keep_unused=True)
    entry = (sharded, in_names, out_names, out_avals, zero_shapes)
    _EXEC_CACHE[key] = entry
    return entry


def run_spmd_cached(nc, in_maps):
    import jax
    sharded, in_names, out_names, out_avals, zero_shapes = _get_executor(nc)
    n_cores = len(in_maps)
    concat_in = [
        np.concatenate([np.asarray(in_maps[c][nm])[None] for c in range(n_cores)],
                       axis=0).reshape(n_cores * np.asarray(in_maps[0][nm]).shape[0],
                                       *np.asarray(in_maps[0][nm]).shape[1:])
        for nm in in_names
    ]
    concat_zeros = [np.zeros((n_cores * sh[0], *sh[1:]), dt)
                    for sh, dt in zero_shapes]
    out_arrs = sharded(*concat_in, *concat_zeros)
    res = []
    for c in range(n_cores):
        res.append({nm: np.asarray(out_arrs[i]).reshape(
            n_cores, *out_avals[i].shape)[c] for i, nm in enumerate(out_names)})
    return res


def prep_core_inputs(inputs, lo, b_core):
    """Host-side shard prep with DMA-friendly swizzles for [lo, lo+b_core)."""
    CH, NT = 512, 4
    nchunks = b_core // CH
    hc = np.asarray(inputs["hand_cards"])[lo:lo + b_core]
    gs = np.asarray(inputs["game_state"])[lo:lo + b_core]
    hs = np.asarray(inputs["hand_size"])[lo:lo + b_core]
    cards_sw = np.ascontiguousarray(
        hc.reshape(nchunks, NT, 128, S).transpose(0, 2, 1, 3)
        .reshape(nchunks * 128, NT * S).astype(np.int32))
    gs_t = np.ascontiguousarray(gs.T.astype(BF16))
    rl_row = np.ascontiguousarray(
        (1.0 / np.maximum(hs, 1)).astype(np.float32).reshape(nchunks, CH))
    return dict(hand_cards=cards_sw, game_state=gs_t, rl_row=rl_row)


def unswizzle_out(out_sw, b_core):
    CH, NT = 512, 4
    nchunks = b_core // CH
    return np.ascontiguousarray(
        out_sw.reshape(nchunks, 128, NT, A).transpose(0, 2, 1, 3)
        .reshape(b_core, A))


def build_in_maps(inputs, tables=None):
    if tables is None:
        tables = make_tables(inputs)
    in_maps = []
    for c in range(NCORES):
        m = dict(tables)
        m.update(prep_core_inputs(inputs, c * BC, BC))
        in_maps.append(m)
    return in_maps


def kernel(**inputs):
    nva = int(inputs["num_valid_actions"])
    nc = _get_nc(BC, nva)
    in_maps = build_in_maps(inputs)
    res = run_spmd_cached(nc, in_maps)
    out = np.concatenate(
        [unswizzle_out(res[c]["out"], BC) for c in range(NCORES)], axis=0)
    return out.astype(np.float32)


# revision 26
# speedup vs baseline: 1.3392x; 1.3392x over previous
"""CardAwarePolicy Trainium2 kernel (8-core data parallel), v2.

Design notes:
- All weight-derived tables are precomputed host-side (numpy) in kernel()
  and passed to the device as extra DRAM parameters; the device does only
  the per-sample work.
- Attention collapses into card-id space (q/k/v of a card depend only on
  its id).  Per-sample card histogram h[c] is computed ON THE PE via an
  exact one-hot matmul: 1[x==c] = relu(1 - (xh-ch)^2 - (xl-cl)^2) where
  x = 8*xh + xl (3-bit digit split keeps every product an integer <= 98,
  exact in bf16).  Features [1, xh, xh^2, xl, xl^2] per position are
  transposed once per 128-batch tile; 4 matmuls (2 positions x 64 cards
  packed into 128 partitions) + relu + a summing matmul give h.
- Attention with UNSCALED h: den[ci] = sum_{cj!=0} E[ci,cj] h[cj] (zeroed
  rows in the den table), W = h * recip(den)  (the 1/len cancels), S with
  the cj=0 column zeroed in the S tables, U = S * (h * rlT) where
  rlT[p,b] = 1/max(hand_size,1) is materialized by a rank-1 matmul.
  An augmented 65th value row carries (8/len)*out_b through the fused
  (c1_w @ out_w) projection.
- Then feature-major MLPs, and the 20-action scorer:
  h1 = relu(P1 + Ptab[a]) (bf16, DVE 2x mode / ACT), block-diag s2
  matmuls, and s3 as 5 accumulating matmuls with a [128,32] zero-padded
  w3 selector (output [32, 512] scores.T, PE-transposed back).
- Nothing in the main loop runs on GPSIMD (its per-op overhead measured
  ~2.9us here).  ScalarE uses only {reciprocal, relu, copy} = one table
  set, so no ACT table reloads inside the loop.
"""
import sys

if "/opt/trn_rl_repo" not in sys.path:
    sys.path.insert(0, "/opt/trn_rl_repo")

import numpy as np
import ml_dtypes
from contextlib import ExitStack

import concourse.bass as bass
import concourse.tile as tile
from concourse import mybir

F32 = mybir.dt.float32
BF = mybir.dt.bfloat16
I32 = mybir.dt.int32
R = mybir.dt.float32r
OP = mybir.AluOpType
AF = mybir.ActivationFunctionType

B, S, A, KC = 32768, 8, 20, 4
E, H, D = 64, 4, 16
NCARD = 54
NCORES = 8
BC = B // NCORES  # 4096 per core
NC2 = 64          # padded card space
BF16 = ml_dtypes.bfloat16


def legalize_multiwait(nc):
    """Split >1 sem waits on Drain/CTRL instructions (walrus limit) into
    preceding single-wait EventSemaphore carriers."""
    for fn in nc.m.functions:
        for blk in fn.blocks:
            new_list = []
            for inst in blk.instructions:
                si = inst.sync_info
                if si and si.on_wait and len(si.on_wait) > 1:
                    waits = list(si.on_wait)
                    for w in waits[:-1]:
                        nm = f"{inst.name}-wsplit-{w.id}"
                        d = mybir.InstEventSemaphore(name=nm, ins=[], outs=[])
                        d.engine = inst.engine
                        d.sync_info = mybir.SyncInfo(on_wait=[w], on_update=[])
                        nc.register_instruction(d, overwrite=True)
                        new_list.append(d)
                    si.on_wait[:] = [waits[-1]]
                new_list.append(inst)
            blk.instructions[:] = new_list
    return nc


def make_tables(inputs):
    """Host-side (numpy) computation of every weight-derived constant."""
    f32 = np.float32
    g = lambda k: np.asarray(inputs[k], f32)
    emb, ipw, ipb = g("emb"), g("in_proj_w"), g("in_proj_b")
    ow, ob = g("out_w"), g("out_b")
    g1w, g1b = g("g1_w"), g("g1_b")
    g2w, g2b = g("g2_w"), g("g2_b")
    c1w, c1b = g("c1_w"), g("c1_b")
    c2w, c2b = g("c2_w"), g("c2_b")
    s1w, s1b = g("s1_w"), g("s1_b")
    s2w, s2b = g("s2_w"), g("s2_b")
    s3w, s3b = g("s3_w"), g("s3_b")
    aci = np.asarray(inputs["action_card_indices"], np.int64)
    acc = np.asarray(inputs["action_card_counts"], np.int64)

    embp = np.zeros((NC2, E), f32)
    embp[:NCARD] = emb
    q = embp @ ipw[0:64].T + ipb[0:64]       # [64c, 64]
    k = embp @ ipw[64:128].T + ipb[64:128]
    v = embp @ ipw[128:192].T + ipb[128:192]

    Eh = np.zeros((H, NC2, NC2), f32)        # [h, ci, cj]
    for h in range(H):
        sc = (q[:, h * D:(h + 1) * D] @ k[:, h * D:(h + 1) * D].T) / np.sqrt(
            np.float32(D))
        Eh[h] = np.exp(sc)

    # den tables: DenT[g][cj, u*64+ci] = Eh[2g+u][ci, cj]; zero cj=0, cj>=54
    DenT = np.zeros((NC2, 2, 128), f32)
    for gg in range(2):
        for u in range(2):
            DenT[:, gg, u * 64:(u + 1) * 64] = Eh[2 * gg + u].T
    DenT[0, :, :] = 0.0
    DenT[NCARD:, :, :] = 0.0
    DenT = DenT.reshape(NC2, 256)

    # S tables (bf16, E-1 shift): BD1z ones-blocks + SE blockdiag(E-1);
    # total column (u, cj=0) must be zero -> zero it in both.
    BD1z = np.zeros((128, 128), f32)
    SE = np.zeros((128, 2, 128), f32)
    for u in range(2):
        BD1z[u * 64:(u + 1) * 64, u * 64:(u + 1) * 64] = 1.0
        BD1z[:, u * 64] = 0.0
        for gg in range(2):
            SE[u * 64:(u + 1) * 64, gg, u * 64:(u + 1) * 64] = Eh[2 * gg + u] - 1.0
            SE[:, gg, u * 64] = 0.0
    SE = SE.reshape(128, 256)

    # BDVP[g] [128, 65]: rows (u,cj) -> v cols of head 2g+u; aug col 64
    BDVP = np.zeros((128, 2, E + 1), f32)
    for gg in range(2):
        for u in range(2):
            hh = 2 * gg + u
            BDVP[u * 64:(u + 1) * 64, gg, hh * D:(hh + 1) * D] = \
                v[:, hh * D:(hh + 1) * D]
    BDVP[0:64, 0, E] = 1.0
    BDVP = BDVP.reshape(128, 2 * (E + 1))

    # fused (c1_w[:, :64] @ out_w_aug): hgw [97, 128]
    c1A = c1w[:, 0:64]
    M1T = np.zeros((E + 1, 128), f32)
    M1T[0:E] = (c1A @ ow).T
    M1T[E] = c1A @ ob
    c1wB = c1w[:, 64:96].T                                  # [32, 128]

    # one-hot quadratic tables CP [4 pairs, 33, 128] (bf16-exact ints)
    cs = np.arange(NC2)
    ch, cl = cs >> 3, cs & 7
    CP = np.zeros((4, 33, 128), f32)
    for p in range(4):
        for u in range(2):
            j = 2 * p + u
            col = slice(u * 64, (u + 1) * 64)
            CP[p, 0, col] = 1.0 - ch * ch - cl * cl
            CP[p, 1 + j, col] = 2.0 * ch
            CP[p, 9 + j, col] = -1.0
            CP[p, 17 + j, col] = 2.0 * cl
            CP[p, 25 + j, col] = -1.0
    CPt = CP.transpose(1, 0, 2).reshape(33, 512)

    ones22I = np.tile(np.eye(NC2, dtype=f32), (2, 2))       # [128, 128]
    onesrow = np.ones((1, 128), f32)
    # merged S table (single matmul per g): blockdiag(E) with cj=0 col zeroed
    SEF = np.zeros((128, 2, 128), f32)
    for u in range(2):
        for gg in range(2):
            SEF[u * 64:(u + 1) * 64, gg, u * 64:(u + 1) * 64] = Eh[2 * gg + u]
            SEF[:, gg, u * 64] = 0.0
    SEF = SEF.reshape(128, 256)

    # actions
    ae = embp[np.clip(aci, 0, NC2 - 1)]                     # [A, KC, E]
    amask = (np.arange(KC)[None, :] < acc[:, None]).astype(f32)
    arep = (ae * amask[..., None]).sum(axis=1) / np.maximum(acc, 1)[:, None]
    Ptab = s1w[:, 128:192] @ arep.T + s1b[:, None]          # [64, A]
    Ptab2 = np.zeros((128, A // 2), f32)
    Ptab2[0:64] = Ptab[:, 0::2]
    Ptab2[64:128] = Ptab[:, 1::2]

    s1A = s1w[:, 0:128]                                     # [64, 128]
    s1wAd = np.concatenate([s1A.T, s1A.T], axis=1)          # [128, 128]

    BDs2 = np.zeros((128, 2, 128), f32)
    s2wT = s2w.T                                            # [64, 32]
    for half in range(2):
        BDs2[0:64, half, half * 64:half * 64 + 32] = s2wT
        BDs2[64:128, half, half * 64 + 32:half * 64 + 64] = s2wT
    BDs2 = BDs2.reshape(128, 256)

    W3 = np.zeros((5, 128, 32), f32)
    for jj in range(5):
        for u in range(4):
            W3[jj, u * 32:(u + 1) * 32, 4 * jj + u] = s3w[0]
    W3t = W3.transpose(1, 0, 2).reshape(128, 160)

    biases = np.zeros((128, 8), f32)
    biases[0:64, 0] = g1b
    biases[0:32, 1] = g2b
    biases[:, 2] = c1b
    biases[:, 3] = c2b
    biases[:, 4] = np.tile(s2b, 4)
    biases[:, 5] = s3b[0]

    bf = lambda x: np.ascontiguousarray(x.astype(BF16))
    fl = lambda x: np.ascontiguousarray(x.astype(f32))
    return dict(
        identF=fl(np.eye(128)),
        identB=bf(np.eye(128)),
        cpt=bf(CPt),
        ones22I=bf(ones22I),
        onesrow=fl(onesrow),
        dent=bf(DenT),
        bd1z=bf(BD1z),
        se=bf(SE),
        sef=bf(SEF),
        bdvp=bf(BDVP),
        m1t=bf(M1T),
        c1wB=bf(c1wB),
        g1wT=bf(g1w.T),
        g2wT=bf(g2w.T),
        c2wT=bf(c2w.T),
        s1wAd=bf(s1wAd),
        ptab2f=fl(Ptab2),
        ptab2b=bf(Ptab2),
        bds2=bf(BDs2),
        s2wTt=bf(np.concatenate([s2wT, s2wT], axis=0)),
        w3t=fl(W3t),
        biases=fl(biases),
    )


TABLE_SPECS = [
    ("identF", [128, 128], F32), ("identB", [128, 128], BF),
    ("cpt", [33, 512], BF), ("ones22I", [128, 128], BF),
    ("onesrow", [1, 128], F32), ("dent", [64, 256], BF),
    ("bd1z", [128, 128], BF), ("se", [128, 256], BF),
    ("sef", [128, 256], BF),
    ("bdvp", [128, 130], BF), ("m1t", [65, 128], BF),
    ("c1wB", [32, 128], BF),
    ("g1wT", [12, 64], BF), ("g2wT", [64, 32], BF),
    ("c2wT", [128, 128], BF), ("s1wAd", [128, 128], BF),
    ("ptab2f", [128, 10], F32), ("ptab2b", [128, 10], BF),
    ("bds2", [128, 256], BF), ("s2wTt", [128, 32], BF),
    ("w3t", [128, 160], F32),
    ("biases", [128, 8], F32),
]


def build_nc(b_core=BC, nva=A, reps=1, n_h1_act=1, n_h2_act=4, oh_act=2,
             se_merge=True):
    CH = 512
    assert b_core % CH == 0
    nchunks = b_core // CH
    NT = CH // 128

    nc = bass.Bass()
    dp = nc.declare_dram_parameter
    # host-swizzled layouts: every DMA is a contiguous [128, X] block
    cards_d = dp("hand_cards", [b_core // NT, NT * S], I32, isOutput=False)
    gs_d = dp("game_state", [12, b_core], BF, isOutput=False)
    rl_d = dp("rl_row", [b_core // CH, CH], F32, isOutput=False)
    tbl_d = {name: dp(name, shape, dt, isOutput=False)
             for name, shape, dt in TABLE_SPECS}
    out_d = dp("out", [b_core // 512 * A, 512], F32, isOutput=True)

    with tile.TileContext(nc) as tc:
        with ExitStack() as ctx:
            const = ctx.enter_context(tc.tile_pool(name="const", bufs=1))

            _dmae = [nc.sync, nc.scalar, nc.gpsimd]
            _dmac = [0]

            def cdma(**kw):
                e = _dmae[_dmac[0] % len(_dmae)]
                _dmac[0] += 1
                e.dma_start(**kw)

            # ---- constants (R tiles are DMA'd through a f32 bitcast) ----
            ct = {}
            r_tiles = {"onesrow", "w3t"}
            for name, shape, dt in TABLE_SPECS:
                if name in r_tiles:
                    stage = const.tile(shape, F32, name="s_" + name,
                                       tag="s_" + name)
                    cdma(out=stage, in_=tbl_d[name][:, :])
                    t = const.tile(shape, R, name="c_" + name, tag="c_" + name)
                    nc.vector.tensor_copy(t, stage)
                else:
                    t = const.tile(shape, dt, name="c_" + name, tag="c_" + name)
                    cdma(out=t, in_=tbl_d[name][:, :])
                ct[name] = t
            identF = ct["identF"]
            identB = ct["identB"]

            # ---- per-chunk pipeline ----
            sb = ctx.enter_context(tc.tile_pool(name="sb", bufs=3))
            sbh = ctx.enter_context(tc.tile_pool(name="sbh", bufs=4))
            sbo = ctx.enter_context(tc.tile_pool(name="sbo", bufs=4))
            sbh2 = ctx.enter_context(tc.tile_pool(name="sbh2", bufs=7))
            ps1 = ctx.enter_context(tc.tile_pool(name="ps1", bufs=3, space="PSUM"))
            psACC = ctx.enter_context(tc.tile_pool(name="psACC", bufs=3, space="PSUM"))
            psM = ctx.enter_context(tc.tile_pool(name="psM", bufs=2, space="PSUM"))

            cards_r = cards_d.rearrange("(c p) x -> c p x", p=128)
            gsT_r = gs_d.rearrange("f (c b) -> c f b", b=CH)
            out_r = out_d.rearrange("(c a) b -> c a b", a=A)

            def stageA(cix):
                """loads + histogram + attention + g-MLP -> (hsum, g2s)."""
                # ---------- loads ----------
                c4 = sb.tile([128, NT, S], I32, tag="c4")
                nc.sync.dma_start(out=c4.rearrange("p t s -> p (t s)"),
                                  in_=cards_r[cix])
                gsT = sb.tile([12, CH], BF, tag="gsT")
                nc.sync.dma_start(out=gsT, in_=gsT_r[cix])
                rlrow_f = sb.tile([1, CH], F32, tag="rlrow_f")
                nc.scalar.dma_start(out=rlrow_f, in_=rl_d[cix].unsqueeze(0))

                # ---------- 1/len row -> rlT [128, CH] ----------
                rlrow = sb.tile([1, CH], R, tag="rlrow")
                nc.vector.tensor_copy(rlrow, rlrow_f)
                rlT_ps = ps1.tile([128, CH], F32, tag="p1", name="rlT_ps")
                nc.tensor.matmul(rlT_ps, ct["onesrow"], rlrow, start=True, stop=True)
                rlT = sbh.tile([128, CH], R, tag="rlT")
                nc.vector.tensor_copy(rlT, rlT_ps)

                # ---------- one-hot features ----------
                xh = sb.tile([128, NT, S], I32, tag="xh")
                nc.vector.tensor_scalar(xh, c4, 3, None, OP.arith_shift_right)
                xl = sb.tile([128, NT, S], I32, tag="xl")
                nc.vector.tensor_scalar(xl, c4, 7, None, OP.bitwise_and)
                Fall = sb.tile([128, NT, 33], BF, tag="Fall")
                nc.vector.memset(Fall[:, :, 0:1], 1.0)
                nc.vector.tensor_copy(Fall[:, :, 1:9], xh)
                nc.vector.tensor_copy(Fall[:, :, 17:25], xl)
                nc.vector.tensor_tensor(out=Fall[:, :, 9:17], in0=Fall[:, :, 1:9],
                                        in1=Fall[:, :, 1:9], op=OP.mult)
                nc.vector.tensor_tensor(out=Fall[:, :, 25:33], in0=Fall[:, :, 17:25],
                                        in1=Fall[:, :, 17:25], op=OP.mult)
                F_ps = ps1.tile([33, CH], BF, tag="p1", name="F_ps")
                for t in range(NT):
                    nc.tensor.transpose(F_ps[:, t * 128:(t + 1) * 128],
                                        Fall[:, t, :], identB)
                F_s = sbh.tile([33, CH], BF, tag="F_s")
                nc.vector.tensor_copy(F_s, F_ps)

                # ---------- histogram: 4 pair-matmuls + relu + sum ----------
                h2_ps = psACC.tile([128, CH], F32, tag="pA", name="h2_ps")
                OHs = []
                for p in range(4):
                    OH_ps = ps1.tile([128, CH], F32, tag="p1", name=f"OH{p}")
                    nc.tensor.matmul(OH_ps, ct["cpt"][:, p * 128:(p + 1) * 128],
                                     F_s, start=True, stop=True)
                    OH_s = sbh.tile([128, CH], BF, tag="OH_s", name=f"OHs{p}")
                    if p < oh_act:
                        nc.scalar.activation(OH_s, OH_ps, AF.Relu)
                    else:
                        nc.vector.tensor_scalar_max(OH_s, OH_ps, 0.0)
                    OHs.append(OH_s)
                for p in range(4):
                    nc.tensor.matmul(h2_ps, ct["ones22I"], OHs[p],
                                     start=(p == 0), stop=(p == 3))
                h2s = sbh.tile([128, CH], BF, tag="h2s")
                nc.scalar.activation(h2s, h2_ps, AF.Copy)
                h2nzs = sbh.tile([128, CH], R, tag="h2nzs")
                nc.vector.tensor_tensor(out=h2nzs, in0=h2_ps,
                                        in1=rlT.bitcast(F32), op=OP.mult)

                # ---------- attention in card space ----------
                hs_ps = psACC.tile([E + 1, CH], F32, tag="pA", name="hs_ps")
                for g in range(2):
                    den_ps = ps1.tile([128, CH], F32, tag="p1", name=f"den{g}")
                    nc.tensor.matmul(den_ps, ct["dent"][:, g * 128:(g + 1) * 128],
                                     h2s[0:64, :], start=True, stop=True)
                    lden = sbh.tile([128, CH], F32, tag="lden")
                    nc.scalar.activation(lden, den_ps, AF.Ln)
                    rden = sbh.tile([128, CH], F32, tag="rden")
                    nc.scalar.activation(rden, lden, AF.Exp, scale=-1.0)
                    Wg = sbh.tile([128, CH], BF, tag="Wg")
                    nc.vector.tensor_tensor(out=Wg, in0=rden,
                                            in1=h2s, op=OP.mult)
                    S_ps = ps1.tile([128, CH], F32, tag="p1", name=f"S{g}")
                    if se_merge:
                        nc.tensor.matmul(S_ps, ct["sef"][:, g * 128:(g + 1) * 128],
                                         Wg, start=True, stop=True)
                    else:
                        nc.tensor.matmul(S_ps, ct["bd1z"], Wg, start=True,
                                         stop=False)
                        nc.tensor.matmul(S_ps, ct["se"][:, g * 128:(g + 1) * 128],
                                         Wg, start=False, stop=True)
                    Ug = sbh.tile([128, CH], BF, tag="Ug")
                    nc.vector.tensor_tensor(out=Ug, in0=S_ps,
                                            in1=h2nzs.bitcast(F32), op=OP.mult)
                    nc.tensor.matmul(hs_ps, ct["bdvp"][:, g * 65:(g + 1) * 65],
                                     Ug, start=(g == 0), stop=(g == 1))

                # ---------- g-MLP + handsum evac ----------
                bia = ct["biases"]
                hsum = sbh.tile([65, CH], BF, tag="hsum")
                nc.scalar.activation(hsum, hs_ps, AF.Copy)
                g1_ps = psM.tile([64, CH], F32, tag="pM", name="g1_ps")
                nc.tensor.matmul(g1_ps, ct["g1wT"], gsT, start=True, stop=True)
                g1s = sbh.tile([64, CH], BF, tag="g1s")
                nc.scalar.activation(g1s, g1_ps, AF.Relu, bias=bia[0:64, 0:1])
                g2_ps = psM.tile([32, CH], F32, tag="pM", name="g2_ps")
                nc.tensor.matmul(g2_ps, ct["g2wT"], g1s, start=True, stop=True)
                g2s = sbh.tile([32, CH], BF, tag="g2s")
                nc.scalar.activation(g2s, g2_ps, AF.Relu, bias=bia[0:32, 1:2])
                return cix, hsum, g2s

            def stageB(state):
                """context MLP + action scorer + store for a prior chunk."""
                cix, hsum, g2s = state
                bia = ct["biases"]
                c1_ps = psM.tile([128, CH], F32, tag="pM", name="c1_ps")
                nc.tensor.matmul(c1_ps, ct["m1t"], hsum, start=True, stop=False)
                nc.tensor.matmul(c1_ps, ct["c1wB"], g2s, start=False, stop=True)
                ctx1 = sbh.tile([128, CH], BF, tag="ctx1")
                nc.scalar.activation(ctx1, c1_ps, AF.Relu, bias=bia[:, 2:3])
                c2_ps = psM.tile([128, CH], F32, tag="pM", name="c2_ps")
                nc.tensor.matmul(c2_ps, ct["c2wT"], ctx1, start=True, stop=True)
                ctx2 = sbh.tile([128, CH], BF, tag="ctx2")
                nc.scalar.activation(ctx2, c2_ps, AF.Relu, bias=bia[:, 3:4])
                p1_ps = psM.tile([128, CH], F32, tag="pM", name="p1_ps")
                nc.tensor.matmul(p1_ps, ct["s1wAd"], ctx2, start=True, stop=True)
                P1d = sbh.tile([128, CH], BF, tag="P1d")
                nc.vector.tensor_copy(P1d, p1_ps)

                # ---------- actions ----------
                scST_ps = psACC.tile([32, CH], F32, tag="pA", name="scST_ps")
                h1s = []
                for j in range(10):
                    h1 = sbh2.tile([128, CH], BF, tag="h1", name=f"h1_{j}")
                    if (j % 10) < n_h1_act:
                        nc.scalar.activation(h1, P1d, AF.Relu,
                                             bias=ct["ptab2f"][:, j:j + 1])
                    else:
                        nc.vector.tensor_scalar(
                            h1, P1d, ct["ptab2f"][:, j:j + 1], 0.0,
                            OP.add, OP.max)
                    h1s.append(h1)
                for jj in range(5):
                    hp = psM.tile([128, CH], F32, tag="pM", name=f"hp{jj}")
                    for half in range(2):
                        nc.tensor.matmul(hp,
                                         ct["bds2"][:, half * 128:(half + 1) * 128],
                                         h1s[jj * 2 + half],
                                         start=(half == 0), stop=(half == 1))
                    h2t = sbh2.tile([128, CH], R, tag="h2t", name=f"h2t{jj}")
                    if jj < n_h2_act:
                        nc.scalar.activation(h2t, hp, AF.Relu, bias=bia[:, 4:5])
                    else:
                        nc.vector.tensor_scalar(h2t, hp,
                                                bia[:, 4:5], 0.0, OP.add, OP.max)
                    nc.tensor.matmul(scST_ps, ct["w3t"][:, jj * 32:(jj + 1) * 32],
                                     h2t, start=(jj == 0), stop=(jj == 4))
                scS = sbo.tile([32, CH], F32, tag="scS")
                nc.scalar.activation(scS, scST_ps, AF.Copy)
                nc.scalar.dma_start(out=out_r[cix], in_=scS[0:A, :])

            # software-pipelined emission: chunk c's stageA interleaves with
            # chunk c-1's stageB so the PE always has independent work
            prev = None
            for cix in [c for _ in range(reps) for c in range(nchunks)]:
                st = stageA(cix)
                if prev is not None:
                    stageB(prev)
                prev = st
            stageB(prev)

    legalize_multiwait(nc)
    return nc


_NC_CACHE = {}


def _get_nc(b_core, nva):
    key = (b_core, nva)
    if key not in _NC_CACHE:
        _NC_CACHE[key] = build_nc(b_core=b_core, nva=nva)
    return _NC_CACHE[key]


# ---- cached PJRT execution path (avoids per-call retrace/re-lowering) ----
_EXEC_CACHE = {}


def _get_executor(nc):
    """Build (once) a jitted shard_map callable around the bass_exec
    primitive for `nc`, so repeated kernel() calls skip retracing and
    re-lowering.  Mirrors concourse.bass2jax.run_bass_via_pjrt."""
    key = id(nc)
    if key in _EXEC_CACHE:
        return _EXEC_CACHE[key]
    import jax
    from jax.sharding import Mesh, PartitionSpec
    from jax.experimental.shard_map import shard_map
    from concourse import bass2jax
    from concourse.bass2jax import _bass_exec_p, install_neuronx_cc_hook

    install_neuronx_cc_hook()
    partition_name = (nc.partition_id_tensor.name
                      if nc.partition_id_tensor else None)
    in_names, out_names, out_avals, zero_shapes = [], [], [], []
    for alloc in nc.m.functions[0].allocations:
        if not isinstance(alloc, mybir.MemoryLocationSet):
            continue
        name = alloc.memorylocations[0].name
        if alloc.kind == "ExternalInput":
            if name != partition_name:
                in_names.append(name)
        elif alloc.kind == "ExternalOutput":
            out_names.append(name)
            shape = tuple(alloc.tensor_shape)
            dtype = mybir.dt.np(alloc.dtype)
            out_avals.append(jax.core.ShapedArray(shape, dtype))
            zero_shapes.append((shape, dtype))
    n_params = len(in_names)
    n_outs = len(out_names)
    all_names = in_names + out_names
    if partition_name is not None:
        all_names = all_names + [partition_name]
    donate = tuple(range(n_params, n_params + n_outs))

    def _body(*args):
        operands = list(args)
        if partition_name is not None:
            operands.append(bass2jax.partition_id_tensor())
        outs = _bass_exec_p.bind(
            *operands,
            out_avals=tuple(out_avals),
            in_names=tuple(all_names),
            out_names=tuple(out_names),
            lowering_input_output_aliases=(),
            sim_require_finite=True,
            sim_require_nnan=True,
            nc=nc,
        )
        return tuple(outs)

    devices = jax.devices()[:NCORES]
    mesh = Mesh(np.asarray(devices), ("core",))
    in_specs = (PartitionSpec("core"),) * (n_params + n_outs)
    out_specs = (PartitionSpec("core"),) * n_outs
    sharded = jax.jit(
        shard_map(_body, mesh=mesh, in_specs=in_specs, out_specs=out_specs,
                  check_rep=False),
        donate_argnums=donate, keep_unused=True)
    entry = (sharded, in_names, out_names, out_avals, zero_shapes)
    _EXEC_CACHE[key] = entry
    return entry


def run_spmd_cached(nc, in_maps):
    import jax
    sharded, in_names, out_names, out_avals, zero_shapes = _get_executor(nc)
    n_cores = len(in_maps)
    concat_in = [
        np.concatenate([np.asarray(in_maps[c][nm])[None] for c in range(n_cores)],
                       axis=0).reshape(n_cores * np.asarray(in_maps[0][nm]).shape[0],
                                       *np.asarray(in_maps[0][nm]).shape[1:])
        for nm in in_names
    ]
    concat_zeros = [np.zeros((n_cores * sh[0], *sh[1:]), dt)
                    for sh, dt in zero_shapes]
    out_arrs = sharded(*concat_in, *concat_zeros)
    res = []
    for c in range(n_cores):
        res.append({nm: np.asarray(out_arrs[i]).reshape(
            n_cores, *out_avals[i].shape)[c] for i, nm in enumerate(out_names)})
    return res


def prep_core_inputs(inputs, lo, b_core):
    """Host-side shard prep with DMA-friendly swizzles for [lo, lo+b_core)."""
    CH, NT = 512, 4
    nchunks = b_core // CH
    hc = np.asarray(inputs["hand_cards"])[lo:lo + b_core]
    gs = np.asarray(inputs["game_state"])[lo:lo + b_core]
    hs = np.asarray(inputs["hand_size"])[lo:lo + b_core]
    cards_sw = np.ascontiguousarray(
        hc.reshape(nchunks, NT, 128, S).transpose(0, 2, 1, 3)
        .reshape(nchunks * 128, NT * S).astype(np.int32))
    gs_t = np.ascontiguousarray(gs.T.astype(BF16))
    rl_row = np.ascontiguousarray(
        (1.0 / np.maximum(hs, 1)).astype(np.float32).reshape(nchunks, CH))
    return dict(hand_cards=cards_sw, game_state=gs_t, rl_row=rl_row)


def unswizzle_out(out_sw, b_core, s3b=0.0, nva=A):
    CH = 512
    nchunks = b_core // CH
    out = (out_sw.reshape(nchunks, A, CH).transpose(0, 2, 1)
           .reshape(b_core, A) + np.float32(s3b))
    if nva < A:
        out[:, nva:] = np.float32(-1e8)
    return np.ascontiguousarray(out.astype(np.float32))


def build_in_maps(inputs, tables=None):
    if tables is None:
        tables = make_tables(inputs)
    in_maps = []
    for c in range(NCORES):
        m = dict(tables)
        m.update(prep_core_inputs(inputs, c * BC, BC))
        in_maps.append(m)
    return in_maps


def kernel(**inputs):
    nva = int(inputs["num_valid_actions"])
    nc = _get_nc(BC, nva)
    in_maps = build_in_maps(inputs)
    res = run_spmd_cached(nc, in_maps)
    s3b = float(np.asarray(inputs["s3_b"]).reshape(-1)[0])
    out = np.concatenate(
        [unswizzle_out(res[c]["out"], BC, s3b, nva) for c in range(NCORES)],
        axis=0)
    return out.astype(np.float32)
